# revision 38
# baseline (speedup 1.0000x reference)
"""Trainium2 Bass kernel V3 for a 2-layer BiLSTM + FC + exp.

B=64, T=1024, D=256, H=256/dir, O=64; data-parallel batch over 8 cores.

V3 vs V2: the step period is bound by the serial cross-engine chain
(ACT tanh -> DVE cell ops -> ACT tanh(c) -> DVE h), not PE weight loads.
Chain shrunk from 7 to 5 links / 3 DVE ops per dir-step: gate chunks
reordered host-side to (g, f, i, o) with c_prev co-located in a
[128, 10, 8] "ctg" tile ahead of the tanh'd gates, so a and b fuse into
ONE scalar_tensor_tensor ab = (ctg[4:8]+1)*ctg[0:4]; c update is one STT
writing into the NEXT step's ctg tile; h~ written for both k-chunks in
one STT; gx thunks rate-limited per step; gx PSUM pool deepened to 3 bufs
so stalled gx matmuls at the PE FIFO head do not block the ladders.

Key ideas vs V1:
  - tanh-only activations: sigma(x) = (tanh(x/2)+1)/2, with the 1/2 folded
    into weight rows host-side; h~=2h and c~=2c kept on-chip, with the 1/2
    folded into all consumers of h (Whh, Wih_l1, fc_W).  One tanh over all
    4 gates + one tanh(c) per step per direction, all servable from the
    single exp_and_others ACT table (which also holds Exp for the FC).
  - gate math on DVE as 4 fused scalar_tensor_tensor ops per step per dir.
  - gx (input projection) computed on-chip in 32-step blocks into SBUF
    (no DRAM round trips), injected into the recurrence PSUM via one wide
    identity matmul per 8-step PSUM block; bias via rank-1 matmul.
  - optional fp8e4 recurrent weights (Whh) to halve PE LDWEIGHTS time.
"""

import os
import sys
from collections import deque
from contextlib import ExitStack

import ml_dtypes
import numpy as np
import orjson

import concourse.bass as bass
import concourse.mybir as mybir
import concourse.tile as tile
from concourse.bass_utils import run_bass_kernel_spmd

# TC_A1 is a leftover per-partition constant tile value (deg-5 tanh poly
# a1 coefficient); the rconst tile using it is retained so the compiled
# program matches the verified build exactly.
TC_A1 = 0.49717735

_LEGALIZE_SKIP = {"EventSemaphore", "UnconditionalBranch", "Call",
                  "ConditionalBranch"}


def _legalize_waits(bir_bytes, limit=1):
    bir = orjson.loads(bir_bytes)
    uid = [0]
    for fn in bir.get("functions") or []:
        for bb in fn.get("blocks") or []:
            insts = bb.get("instructions")
            if not insts:
                continue
            out = []
            for inst in insts:
                si = inst.get("sync_info")
                if si and inst.get("opcode") not in _LEGALIZE_SKIP:
                    waits = si.get("on_wait") or []
                    if len(waits) > limit:
                        for w in waits[:-limit]:
                            uid[0] += 1
                            out.append({
                                "name": f"{inst['name']}_hw{uid[0]}",
                                "opcode": "EventSemaphore",
                                "engine": inst["engine"],
                                "ins": [], "outs": [],
                                "debug": inst.get("debug"),
                                "sync_info": {"on_wait": [w], "on_update": []},
                            })
                        si["on_wait"] = waits[-limit:]
                out.append(inst)
            bb["instructions"] = out
    return orjson.dumps(bir)


def _patch_nc(nc):
    orig = nc.to_json_bytes
    nc.to_json_bytes = lambda: _legalize_waits(orig())
    return nc


F16 = mybir.dt.float16
F32 = mybir.dt.float32
F8 = mybir.dt.float8e4
AF = mybir.ActivationFunctionType
ADD = mybir.AluOpType.add
MULT = mybir.AluOpType.mult

WHH_F8 = os.environ.get("WHH_F8", "1") == "1"

BL = 8          # batch per core
NCORES = 8
D = 256
H = 256
G = 8           # gate chunks of 128
KH = 2          # hidden chunks of 128
RS = 32         # steps per gx round
PB = 8          # steps per PSUM block


def build_nc(T):
    BT = T * BL
    NR = T // RS
    whh_dt = F8 if WHH_F8 else F16
    nc = bass.Bass()

    xT = nc.dram_tensor("xT", [128, 2, BT], F16, kind="ExternalInput")
    wih0 = nc.dram_tensor("wih0", [2, 128, 2, 1024], F16, kind="ExternalInput")
    whh0 = nc.dram_tensor("whh0", [2, 128, 2, 1024], whh_dt, kind="ExternalInput")
    b0 = nc.dram_tensor("b0", [2, 128, 8], F32, kind="ExternalInput")
    wih1 = nc.dram_tensor("wih1", [2, 128, 4, 1024], F16, kind="ExternalInput")
    whh1 = nc.dram_tensor("whh1", [2, 128, 2, 1024], whh_dt, kind="ExternalInput")
    b1 = nc.dram_tensor("b1", [2, 128, 8], F32, kind="ExternalInput")
    fcw = nc.dram_tensor("fcw", [128, 4, 64], F16, kind="ExternalInput")
    fcb = nc.dram_tensor("fcb", [64, 1], F32, kind="ExternalInput")
    ident = nc.dram_tensor("ident", [128, 128], F16, kind="ExternalInput")
    outT = nc.dram_tensor("outT", [64, BT], F32, kind="ExternalOutput")

    with tile.TileContext(nc) as tc, ExitStack() as ctx:
        wpool = ctx.enter_context(tc.tile_pool(name="weights", bufs=1))

        def wtile(name, src, shape, dtype):
            t = wpool.tile(shape, dtype, name=name)
            nc.sync.dma_start(t[:], src)
            return t

        wih_sb = [
            [wtile(f"wih0_{d}", wih0[d], [128, 2, 1024], F16) for d in range(2)],
            [wtile(f"wih1_{d}", wih1[d], [128, 4, 1024], F16) for d in range(2)],
        ]
        whh_sb = [
            [wtile(f"whh0_{d}", whh0[d], [128, 2, 1024], whh_dt) for d in range(2)],
            [wtile(f"whh1_{d}", whh1[d], [128, 2, 1024], whh_dt) for d in range(2)],
        ]
        b_sb = [
            [wtile(f"b0_{d}", b0[d], [128, 8], F32) for d in range(2)],
            [wtile(f"b1_{d}", b1[d], [128, 8], F32) for d in range(2)],
        ]
        fcw_sb = wtile("fcw_sb", fcw[:], [128, 4, 64], F16)
        fcb_sb = wtile("fcb_sb", fcb[:], [64, 1], F32)
        ident_sb = wtile("ident_sb", ident[:], [128, 128], F16)

        # big h~ tiles (layer0 feeds layer1 gx; layer1 feeds FC)
        hb = [[wpool.tile([128, 2, BT], F16, name=f"h{l}_{d}") for d in range(2)]
              for l in range(2)]
        hz = wpool.tile([128, 2, BL], F16, name="hz")
        nc.vector.memset(hz[:], 0.0)
        # a1-coefficient operand for TANH_HALF_ANT, sized to match the
        # [128, 2, BL] tc tile's flattened free dim (Src1 streams elementwise)
        rconst = wpool.tile([128, 2 * BL], F32, name="rconst")
        nc.vector.memset(rconst[:], TC_A1)
        # ctg ping-pong tiles per dir: [:, 0:2] = c~_prev (written by the
        # previous step's c-STT), [:, 2:10] = tanh of the 8 gate chunks in
        # host order (g, f, i, o).
        ctg = [[wpool.tile([128, 10, BL], F32, name=f"ctg{d}_{p}")
                for p in range(2)] for d in range(2)]

        xpool = ctx.enter_context(tc.tile_pool(name="xp", bufs=3))
        fcpool = ctx.enter_context(tc.tile_pool(name="fcp", bufs=2))
        fc_ps = ctx.enter_context(tc.tile_pool(name="fc_ps", bufs=1, space="PSUM"))

        def fc_thunks(tb):
            cr = slice(tb * RS * BL, (tb + 1) * RS * BL)
            th = []
            psf = [None]

            def t_mm(kk):
                def f():
                    if kk == 0:
                        psf[0] = fc_ps.tile([64, 256], F32, tag="fc", name="fcps")
                    nc.tensor.matmul(psf[0][:], fcw_sb[:, kk, :],
                                     hb[1][kk // 2][:, kk % 2, cr],
                                     start=(kk == 0), stop=(kk == 3))
                return f
            for kk in range(4):
                th.append(t_mm(kk))

            def t_act():
                ob = fcpool.tile([64, 256], F32, tag="ob", name="ob")
                nc.scalar.activation(ob[:], psf[0][:], AF.Exp, bias=fcb_sb[:])
                nc.sync.dma_start(outT[:, cr], ob[:])
            th.append(t_act)
            return th

        for layer in range(2):
            nk = 2 if layer == 0 else 4
            with ExitStack() as lctx:
                gxp = lctx.enter_context(tc.tile_pool(name=f"gxl{layer}", bufs=2))
                # bufs=4: a gx matmul WARs against the copy from 3 chunks
                # ago instead of 1 — a stalled gx mm at the PE FIFO head
                # blocks the recurrence ladders behind it
                gxps = lctx.enter_context(
                    tc.tile_pool(name=f"gxps{layer}", bufs=3, space="PSUM"))
                rps = lctx.enter_context(
                    tc.tile_pool(name=f"rps{layer}", bufs=2, space="PSUM"))
                rec = lctx.enter_context(tc.tile_pool(name=f"rec{layer}", bufs=3))

                xblk = {}
                gx_cur = [None, None]
                gx_next = [None, None]
                psrec_next = [None, None]
                # reset c~ for this layer (step 0 reads parity-0 tile)
                for d in range(2):
                    nc.vector.memset(ctg[d][0][:, 0:2, :], 0.0)

                def colrange(d, r):
                    if d == 0:
                        return slice(r * RS * BL, (r + 1) * RS * BL)
                    return slice((T - (r + 1) * RS) * BL, (T - r * RS) * BL)

                def dma_x(d, r):
                    t = xpool.tile([128, 2, 256], F16, tag=f"x{d}", name=f"x{d}")
                    nc.sync.dma_start(t[:], xT[:, :, colrange(d, r)])
                    xblk[d] = t

                def gx_round_thunks(d, r):
                    """emit thunks computing gx block (d, r) into a fresh tile
                    (stored to gx_next[d] on first thunk run)."""
                    cr = colrange(d, r)
                    th = []
                    st = {"ps": None, "gxs": None}

                    def mk_mm(m, k):
                        def f():
                            if st["gxs"] is None:
                                st["gxs"] = gxp.tile([128, G, 256], F16,
                                                     tag=f"gx{d}", name=f"gx{d}")
                                gx_next[d] = st["gxs"]
                            if k == 0:
                                st["ps"] = gxps.tile([128, 256], F32,
                                                     tag="gxps", name="gxps")
                            if layer == 0:
                                mov = xblk[d][:, k, :]
                            else:
                                mov = hb[0][k // 2][:, k % 2, cr]
                            nc.tensor.matmul(st["ps"][:],
                                             wih_sb[layer][d][:, k,
                                                              m * 128:(m + 1) * 128],
                                             mov, start=(k == 0),
                                             stop=(k == nk - 1))
                        return f

                    def mk_copy(m):
                        # bias folded into the PSUM->SBUF copy (GpSimd can't
                        # read PSUM); alternate ACT/DVE to balance their load
                        def f():
                            bias = b_sb[layer][d][:, m:m + 1]
                            if m % 2 == 1:
                                nc.scalar.activation(st["gxs"][:, m, :],
                                                     st["ps"][:], AF.Identity,
                                                     bias=bias)
                            else:
                                nc.vector.tensor_scalar_add(
                                    st["gxs"][:, m, :], st["ps"][:], bias)
                        return f

                    for m in range(G):
                        for k in range(nk):
                            th.append(mk_mm(m, k))
                        th.append(mk_copy(m))
                    return th

                # layer prologue: first gx blocks for both dirs (eager)
                for d in range(2):
                    if layer == 0:
                        dma_x(d, 0)
                    for f in gx_round_thunks(d, 0):
                        f()
                    gx_cur[d] = gx_next[d]

                for r in range(NR):
                    thunks = deque()
                    if r + 1 < NR:
                        for d in range(2):
                            if layer == 0:
                                # direct emit: dma_start is a Sync-engine op,
                                # no need to burn a PE-coupled thunk slot; the
                                # DMA gets a full round of lead time
                                dma_x(d, r + 1)
                            thunks.extend(gx_round_thunks(d, r + 1))
                    if layer == 1 and r >= NR // 2 + 1:
                        # token blocks completed at end of round r-1
                        for tb in {r - 1, NR - r}:
                            thunks.extend(fc_thunks(tb))

                    psrec = [None, None]
                    for s_local in range(RS):
                        s = r * RS + s_local
                        inj = s_local % PB == 0
                        if inj and psrec_next[0] is not None:
                            psrec = psrec_next
                            psrec_next = [None, None]
                            inj = False  # pre-emitted 2 steps ago
                        sbq = s_local % PB
                        ab = [None, None]
                        tc_t = [None, None]
                        toks = [sbq, PB - 1 - sbq]
                        ts = [s, T - 1 - s]
                        p, pn = s % 2, (s + 1) % 2
                        # phase 1: both dirs' matmuls + gate tanh -> ctg[2:10]
                        for d in range(2):
                            if inj:
                                # gx injection emitted per-dir right before
                                # that dir's ladder, so dir0's ladder is not
                                # stuck behind dir1's 585ns inject on the PE
                                q = s_local // PB
                                psrec[d] = rps.tile([128, G, PB * BL], F32,
                                                    tag=f"ps{d}",
                                                    name=f"rps{d}")
                                tq = q if d == 0 else (RS // PB - 1) - q
                                nc.tensor.matmul(
                                    psrec[d][:], ident_sb[:],
                                    gx_cur[d][:, :, tq * 64:(tq + 1) * 64],
                                    start=True, stop=False,
                                    skip_group_check=True)
                            t, tok = ts[d], toks[d]
                            if s == 0:
                                hprev = hz
                            else:
                                tp = (t - 1) if d == 0 else (t + 1)
                                hprev = hb[layer][d][:, :, tp * BL:(tp + 1) * BL]
                            for k in range(KH):
                                for m in range(G):
                                    nc.tensor.matmul(
                                        psrec[d][:, m, tok * BL:(tok + 1) * BL],
                                        whh_sb[layer][d][:, k,
                                                         m * 128:(m + 1) * 128],
                                        hprev[:, k, :],
                                        start=False, stop=(k == KH - 1),
                                        skip_group_check=True)
                            nc.scalar.activation(
                                ctg[d][p][:, 2:10, :],
                                psrec[d][:, :, tok * BL:(tok + 1) * BL],
                                AF.Tanh)
                        # phase 2: fused cell updates on DVE.  gate chunk
                        # order is (g, f, i, o); ctg[0:2] = c~_prev, so
                        # ab = (ctg[4:8]+1)*ctg[0:4]
                        #    = [(f~+1)*c~_prev | (i~+1)*g~] = [a | b]
                        for d in range(2):
                            ab[d] = rec.tile([128, 4, BL], F32, tag=f"ab{d}",
                                             name=f"ab{d}")
                            nc.vector.scalar_tensor_tensor(
                                ab[d][:], ctg[d][p][:, 4:8, :], 1.0,
                                ctg[d][p][:, 0:4, :], ADD, MULT)
                            nc.vector.scalar_tensor_tensor(
                                ctg[d][pn][:, 0:2, :], ab[d][:, 0:2, :], 0.5,
                                ab[d][:, 2:4, :], MULT, ADD)
                        for d in range(2):
                            tc_t[d] = rec.tile([128, 2, BL], F32, tag=f"tc{d}",
                                               name=f"tc{d}")
                            nc.scalar.activation(tc_t[d][:],
                                                 ctg[d][pn][:, 0:2, :],
                                                 AF.Tanh, scale=0.5)
                        # h~ split by k-chunk: the mm ladder is k-outer, so
                        # the k=0 matmuls of step s+1 can start as soon as
                        # the first half of h~ lands.  (GpSimd/Pool rejects
                        # STT in this compiler build, so both stay on DVE.)
                        for d in range(2):
                            for k in range(KH):
                                nc.vector.scalar_tensor_tensor(
                                    hb[layer][d][:, k,
                                                 ts[d] * BL:(ts[d] + 1) * BL],
                                    ctg[d][p][:, 8 + k, :], 1.0,
                                    tc_t[d][:, k, :], ADD, MULT)
                        # gx/FC/DMA thunks drained at END of step so their
                        # ACT/DVE copies queue AFTER this step's chain ops.
                        # Rate-matched to the per-step PE idle gap (~1us):
                        # each mm thunk is a ~400ns matmul, so draining more
                        # than 2-3 per step overruns the gap and delays the
                        # next ladder past h-ready.
                        # gx injects for the NEXT PSUM block are emitted two
                        # steps early so the 585ns matmuls run in these
                        # steps' chain-wait gaps instead of delaying the
                        # block's first ladder (injects depend only on gx)
                        pre = (s_local % PB == PB - 2
                               and s_local // PB < RS // PB - 1)
                        if pre:
                            qq = s_local // PB + 1
                            for d in range(2):
                                psrec_next[d] = rps.tile(
                                    [128, G, PB * BL], F32,
                                    tag=f"ps{d}", name=f"rps{d}")
                                tq = qq if d == 0 else (RS // PB - 1) - qq
                                nc.tensor.matmul(
                                    psrec_next[d][:], ident_sb[:],
                                    gx_cur[d][:, :, tq * 64:(tq + 1) * 64],
                                    start=True, stop=False,
                                    skip_group_check=True)
                        # no drains in the first 2 steps of a round: the
                        # previous round's tail drains are still retiring
                        # and stacking more gx matmuls into the PE FIFO
                        # delays the boundary ladders by ~3us
                        nd = 0 if s_local < 2 else (2 if layer == 0 else 3)
                        for _ in range(nd):
                            if thunks:
                                thunks.popleft()()
                        # cross-round: pre-emit the next round's block-0
                        # injects once this round's thunks have drained, so
                        # the round boundary doesn't serialize on them
                        if (s_local == RS - 2 and r + 1 < NR and not thunks
                                and gx_next[0] is not None):
                            for d in range(2):
                                psrec_next[d] = rps.tile(
                                    [128, G, PB * BL], F32,
                                    tag=f"ps{d}", name=f"rps{d}")
                                tq = 0 if d == 0 else RS // PB - 1
                                nc.tensor.matmul(
                                    psrec_next[d][:], ident_sb[:],
                                    gx_next[d][:, :, tq * 64:(tq + 1) * 64],
                                    start=True, stop=False,
                                    skip_group_check=True)
                    while thunks:
                        thunks.popleft()()
                    for d in range(2):
                        gx_cur[d] = gx_next[d]

        # FC epilogue: blocks finished in the last round
        for tb in (0, NR - 1):
            for f in fc_thunks(tb):
                f()

    return nc


# ---------------------------------------------------------------------------
# host-side preparation
# ---------------------------------------------------------------------------

def _rowscale():
    rs = np.full(4 * H, 0.5, np.float32)
    rs[2 * H:3 * H] = 1.0  # g rows keep scale 1 (direct tanh)
    return rs


# PyTorch gate order is (i, f, g, o) in 128-row chunks (0,1, 2,3, 4,5, 6,7);
# the kernel wants (g, f, i, o) so the fused DVE ab op sees contiguous
# [f, i] and [c, g] operand blocks.
_GPERM = np.array([4, 5, 2, 3, 0, 1, 6, 7])


def _permute_gates(wT):
    """Permute the last (1024-wide gate) axis of wT by _GPERM chunks."""
    shp = wT.shape
    v = wT.reshape(*shp[:-1], 8, 128)[..., _GPERM, :]
    return np.ascontiguousarray(v.reshape(*shp))


def _prep_w(wih, whh, b, din, in_half):
    """-> wihT [128, din/128, 1024] f16, whhT [128, 2, 1024], b [1, 1024] f16"""
    rs = _rowscale()
    kin = din // 128
    xs = 0.5 if in_half else 1.0
    wihT = (np.asarray(wih, np.float32) * rs[:, None] * xs).T
    wihT = _permute_gates(np.ascontiguousarray(wihT)).astype(np.float16)
    wihT = wihT.reshape(kin, 128, 1024).transpose(1, 0, 2)
    whhT = (np.asarray(whh, np.float32) * rs[:, None] * 0.5).T
    whhT = _permute_gates(np.ascontiguousarray(whhT))
    if WHH_F8:
        whhT = whhT.astype(ml_dtypes.float8_e4m3)
    else:
        whhT = whhT.astype(np.float16)
    whhT = whhT.reshape(2, 128, 1024).transpose(1, 0, 2)
    bp = np.ascontiguousarray((np.asarray(b, np.float32) * rs)
                              .reshape(8, 128)[_GPERM].T)  # [128, 8] chunks
    return (np.ascontiguousarray(wihT), np.ascontiguousarray(whhT), bp)


def prep_weight_map(inputs):
    m = {}
    w0 = [_prep_w(inputs["Wih_l0f"], inputs["Whh_l0f"], inputs["b_l0f"], D, False),
          _prep_w(inputs["Wih_l0b"], inputs["Whh_l0b"], inputs["b_l0b"], D, False)]
    w1 = [_prep_w(inputs["Wih_l1f"], inputs["Whh_l1f"], inputs["b_l1f"], 2 * H, True),
          _prep_w(inputs["Wih_l1b"], inputs["Whh_l1b"], inputs["b_l1b"], 2 * H, True)]
    m["wih0"] = np.stack([w0[0][0], w0[1][0]])
    m["whh0"] = np.stack([w0[0][1], w0[1][1]])
    m["b0"] = np.stack([w0[0][2], w0[1][2]])
    m["wih1"] = np.stack([w1[0][0], w1[1][0]])
    m["whh1"] = np.stack([w1[0][1], w1[1][1]])
    m["b1"] = np.stack([w1[0][2], w1[1][2]])
    fcT = (np.asarray(inputs["fc_W"], np.float32) * 0.5).T.astype(np.float16)
    m["fcw"] = np.ascontiguousarray(fcT.reshape(4, 128, 64).transpose(1, 0, 2))
    m["fcb"] = np.asarray(inputs["fc_b"], np.float32).reshape(64, 1)
    m["ident"] = np.eye(128, dtype=np.float16)
    return m


def prep_x_core(x, c, T):
    xs = np.asarray(x[c * BL:(c + 1) * BL, :T]).astype(np.float16)  # [8, T, 256]
    xt = xs.transpose(2, 1, 0).reshape(2, 128, T * BL).transpose(1, 0, 2)
    return np.ascontiguousarray(xt)


def run(inputs, T=1024, cores=None, trace=False):
    inputs = {k: np.asarray(v) for k, v in inputs.items()}
    if cores is None:
        cores = list(range(NCORES))
    nc = _patch_nc(build_nc(T))
    wm = prep_weight_map(inputs)
    in_maps = [dict(wm, xT=prep_x_core(inputs["x"], c, T)) for c in range(len(cores))]
    res = run_bass_kernel_spmd(nc, in_maps, core_ids=cores, trace=trace)
    outs = []
    for r in res.results:
        o = r["outT"].reshape(64, T, BL).transpose(2, 1, 0)  # [8, T, 64]
        outs.append(o)
    full = np.concatenate(outs, axis=0).astype(np.float32)
    return full, res


def kernel(**inputs):
    out, _ = run(inputs, T=1024, cores=list(range(NCORES)))
    return out



# revision 41
# speedup vs baseline: 1.0004x; 1.0004x over previous
"""Trainium2 Bass kernel V3 for a 2-layer BiLSTM + FC + exp.

B=64, T=1024, D=256, H=256/dir, O=64; data-parallel batch over 8 cores.

V3 vs V2: the step period is bound by the serial cross-engine chain
(ACT tanh -> DVE cell ops -> ACT tanh(c) -> DVE h), not PE weight loads.
Chain shrunk from 7 to 5 links / 3 DVE ops per dir-step: gate chunks
reordered host-side to (g, f, i, o) with c_prev co-located in a
[128, 10, 8] "ctg" tile ahead of the tanh'd gates, so a and b fuse into
ONE scalar_tensor_tensor ab = (ctg[4:8]+1)*ctg[0:4]; c update is one STT
writing into the NEXT step's ctg tile; h~ written for both k-chunks in
one STT; gx thunks rate-limited per step; gx PSUM pool deepened to 3 bufs
so stalled gx matmuls at the PE FIFO head do not block the ladders.

Key ideas vs V1:
  - tanh-only activations: sigma(x) = (tanh(x/2)+1)/2, with the 1/2 folded
    into weight rows host-side; h~=2h and c~=2c kept on-chip, with the 1/2
    folded into all consumers of h (Whh, Wih_l1, fc_W).  One tanh over all
    4 gates + one tanh(c) per step per direction, all servable from the
    single exp_and_others ACT table (which also holds Exp for the FC).
  - gate math on DVE as 4 fused scalar_tensor_tensor ops per step per dir.
  - gx (input projection) computed on-chip in 32-step blocks into SBUF
    (no DRAM round trips), injected into the recurrence PSUM via one wide
    identity matmul per 8-step PSUM block; bias via rank-1 matmul.
  - optional fp8e4 recurrent weights (Whh) to halve PE LDWEIGHTS time.
"""

import os
import sys
from collections import deque
from contextlib import ExitStack

import ml_dtypes
import numpy as np
import orjson

import concourse.bass as bass
import concourse.mybir as mybir
import concourse.tile as tile
from concourse.bass_utils import run_bass_kernel_spmd

# TC_A1 is a leftover per-partition constant tile value (deg-5 tanh poly
# a1 coefficient); the rconst tile using it is retained so the compiled
# program matches the verified build exactly.
TC_A1 = 0.49717735

_LEGALIZE_SKIP = {"EventSemaphore", "UnconditionalBranch", "Call",
                  "ConditionalBranch"}


def _legalize_waits(bir_bytes, limit=1):
    bir = orjson.loads(bir_bytes)
    uid = [0]
    for fn in bir.get("functions") or []:
        for bb in fn.get("blocks") or []:
            insts = bb.get("instructions")
            if not insts:
                continue
            out = []
            for inst in insts:
                si = inst.get("sync_info")
                if si and inst.get("opcode") not in _LEGALIZE_SKIP:
                    waits = si.get("on_wait") or []
                    if len(waits) > limit:
                        for w in waits[:-limit]:
                            uid[0] += 1
                            out.append({
                                "name": f"{inst['name']}_hw{uid[0]}",
                                "opcode": "EventSemaphore",
                                "engine": inst["engine"],
                                "ins": [], "outs": [],
                                "debug": inst.get("debug"),
                                "sync_info": {"on_wait": [w], "on_update": []},
                            })
                        si["on_wait"] = waits[-limit:]
                out.append(inst)
            bb["instructions"] = out
    return orjson.dumps(bir)


def _patch_nc(nc):
    orig = nc.to_json_bytes
    nc.to_json_bytes = lambda: _legalize_waits(orig())
    return nc


F16 = mybir.dt.float16
F32 = mybir.dt.float32
F8 = mybir.dt.float8e4
AF = mybir.ActivationFunctionType
ADD = mybir.AluOpType.add
MULT = mybir.AluOpType.mult

WHH_F8 = os.environ.get("WHH_F8", "1") == "1"

BL = 8          # batch per core
NCORES = 8
D = 256
H = 256
G = 8           # gate chunks of 128
KH = 2          # hidden chunks of 128
RS = 32         # steps per gx round
PB = 8          # steps per PSUM block


def build_nc(T):
    BT = T * BL
    NR = T // RS
    whh_dt = F8 if WHH_F8 else F16
    nc = bass.Bass()

    xT = nc.dram_tensor("xT", [128, 2, BT], F16, kind="ExternalInput")
    wih0 = nc.dram_tensor("wih0", [2, 128, 2, 1024], F16, kind="ExternalInput")
    whh0 = nc.dram_tensor("whh0", [2, 128, 2, 1024], whh_dt, kind="ExternalInput")
    b0 = nc.dram_tensor("b0", [2, 128, 8], F32, kind="ExternalInput")
    wih1 = nc.dram_tensor("wih1", [2, 128, 4, 1024], F16, kind="ExternalInput")
    whh1 = nc.dram_tensor("whh1", [2, 128, 2, 1024], whh_dt, kind="ExternalInput")
    b1 = nc.dram_tensor("b1", [2, 128, 8], F32, kind="ExternalInput")
    fcw = nc.dram_tensor("fcw", [128, 4, 64], F16, kind="ExternalInput")
    fcb = nc.dram_tensor("fcb", [64, 1], F32, kind="ExternalInput")
    ident = nc.dram_tensor("ident", [128, 128], F16, kind="ExternalInput")
    outT = nc.dram_tensor("outT", [64, BT], F32, kind="ExternalOutput")

    with tile.TileContext(nc) as tc, ExitStack() as ctx:
        wpool = ctx.enter_context(tc.tile_pool(name="weights", bufs=1))

        def wtile(name, src, shape, dtype):
            t = wpool.tile(shape, dtype, name=name)
            nc.sync.dma_start(t[:], src)
            return t

        wih_sb = [
            [wtile(f"wih0_{d}", wih0[d], [128, 2, 1024], F16) for d in range(2)],
            [wtile(f"wih1_{d}", wih1[d], [128, 4, 1024], F16) for d in range(2)],
        ]
        whh_sb = [
            [wtile(f"whh0_{d}", whh0[d], [128, 2, 1024], whh_dt) for d in range(2)],
            [wtile(f"whh1_{d}", whh1[d], [128, 2, 1024], whh_dt) for d in range(2)],
        ]
        b_sb = [
            [wtile(f"b0_{d}", b0[d], [128, 8], F32) for d in range(2)],
            [wtile(f"b1_{d}", b1[d], [128, 8], F32) for d in range(2)],
        ]
        fcw_sb = wtile("fcw_sb", fcw[:], [128, 4, 64], F16)
        fcb_sb = wtile("fcb_sb", fcb[:], [64, 1], F32)
        ident_sb = wtile("ident_sb", ident[:], [128, 128], F16)

        # big h~ tiles (layer0 feeds layer1 gx; layer1 feeds FC)
        hb = [[wpool.tile([128, 2, BT], F16, name=f"h{l}_{d}") for d in range(2)]
              for l in range(2)]
        hz = wpool.tile([128, 2, BL], F16, name="hz")
        nc.vector.memset(hz[:], 0.0)
        # a1-coefficient operand for TANH_HALF_ANT, sized to match the
        # [128, 2, BL] tc tile's flattened free dim (Src1 streams elementwise)
        rconst = wpool.tile([128, 2 * BL], F32, name="rconst")
        nc.vector.memset(rconst[:], TC_A1)
        # ctg ping-pong tiles per dir: [:, 0:2] = c~_prev (written by the
        # previous step's c-STT), [:, 2:10] = tanh of the 8 gate chunks in
        # host order (g, f, i, o).
        ctg = [[wpool.tile([128, 10, BL], F32, name=f"ctg{d}_{p}")
                for p in range(2)] for d in range(2)]

        xpool = ctx.enter_context(tc.tile_pool(name="xp", bufs=3))
        fcpool = ctx.enter_context(tc.tile_pool(name="fcp", bufs=2))
        fc_ps = ctx.enter_context(tc.tile_pool(name="fc_ps", bufs=1, space="PSUM"))

        def fc_thunks(tb):
            cr = slice(tb * RS * BL, (tb + 1) * RS * BL)
            th = []
            psf = [None]

            def t_mm(kk):
                def f():
                    if kk == 0:
                        psf[0] = fc_ps.tile([64, 256], F32, tag="fc", name="fcps")
                    nc.tensor.matmul(psf[0][:], fcw_sb[:, kk, :],
                                     hb[1][kk // 2][:, kk % 2, cr],
                                     start=(kk == 0), stop=(kk == 3))
                return f
            for kk in range(4):
                th.append(t_mm(kk))

            def t_act():
                ob = fcpool.tile([64, 256], F32, tag="ob", name="ob")
                nc.scalar.activation(ob[:], psf[0][:], AF.Exp, bias=fcb_sb[:])
                nc.sync.dma_start(outT[:, cr], ob[:])
            th.append(t_act)
            return th

        for layer in range(2):
            nk = 2 if layer == 0 else 4
            with ExitStack() as lctx:
                gxp = lctx.enter_context(tc.tile_pool(name=f"gxl{layer}", bufs=2))
                # bufs=4: a gx matmul WARs against the copy from 3 chunks
                # ago instead of 1 — a stalled gx mm at the PE FIFO head
                # blocks the recurrence ladders behind it
                gxps = lctx.enter_context(
                    tc.tile_pool(name=f"gxps{layer}", bufs=3, space="PSUM"))
                rps = lctx.enter_context(
                    tc.tile_pool(name=f"rps{layer}", bufs=2, space="PSUM"))
                rec = lctx.enter_context(tc.tile_pool(name=f"rec{layer}", bufs=3))

                xblk = {}
                gx_cur = [None, None]
                gx_next = [None, None]
                psrec_next = [None, None]
                # reset c~ for this layer (step 0 reads parity-0 tile)
                for d in range(2):
                    nc.vector.memset(ctg[d][0][:, 0:2, :], 0.0)

                def colrange(d, r):
                    if d == 0:
                        return slice(r * RS * BL, (r + 1) * RS * BL)
                    return slice((T - (r + 1) * RS) * BL, (T - r * RS) * BL)

                def dma_x(d, r):
                    t = xpool.tile([128, 2, 256], F16, tag=f"x{d}", name=f"x{d}")
                    nc.sync.dma_start(t[:], xT[:, :, colrange(d, r)])
                    xblk[d] = t

                def gx_round_thunks(d, r):
                    """emit thunks computing gx block (d, r) into a fresh tile
                    (stored to gx_next[d] on first thunk run)."""
                    cr = colrange(d, r)
                    th = []
                    st = {"ps": None, "gxs": None}

                    def mk_mm(m, k):
                        def f():
                            if st["gxs"] is None:
                                st["gxs"] = gxp.tile([128, G, 256], F16,
                                                     tag=f"gx{d}", name=f"gx{d}")
                                gx_next[d] = st["gxs"]
                            if k == 0:
                                st["ps"] = gxps.tile([128, 256], F32,
                                                     tag="gxps", name="gxps")
                            if layer == 0:
                                mov = xblk[d][:, k, :]
                            else:
                                mov = hb[0][k // 2][:, k % 2, cr]
                            nc.tensor.matmul(st["ps"][:],
                                             wih_sb[layer][d][:, k,
                                                              m * 128:(m + 1) * 128],
                                             mov, start=(k == 0),
                                             stop=(k == nk - 1))
                        return f

                    def mk_copy(m):
                        # bias folded into the PSUM->SBUF copy (GpSimd can't
                        # read PSUM); alternate ACT/DVE to balance their load
                        def f():
                            bias = b_sb[layer][d][:, m:m + 1]
                            if m % 2 == 1:
                                nc.scalar.activation(st["gxs"][:, m, :],
                                                     st["ps"][:], AF.Identity,
                                                     bias=bias)
                            else:
                                nc.vector.tensor_scalar_add(
                                    st["gxs"][:, m, :], st["ps"][:], bias)
                        return f

                    for m in range(G):
                        for k in range(nk):
                            th.append(mk_mm(m, k))
                        th.append(mk_copy(m))
                    return th

                # layer prologue: first gx blocks for both dirs (eager)
                for d in range(2):
                    if layer == 0:
                        dma_x(d, 0)
                    for f in gx_round_thunks(d, 0):
                        f()
                    gx_cur[d] = gx_next[d]

                for r in range(NR):
                    thunks = deque()
                    if r + 1 < NR:
                        for d in range(2):
                            if layer == 0:
                                thunks.append(lambda d=d, r=r: dma_x(d, r + 1))
                            thunks.extend(gx_round_thunks(d, r + 1))
                    if layer == 1 and r >= NR // 2 + 1:
                        # token blocks completed at end of round r-1
                        for tb in {r - 1, NR - r}:
                            thunks.extend(fc_thunks(tb))

                    psrec = [None, None]
                    for s_local in range(RS):
                        s = r * RS + s_local
                        inj = s_local % PB == 0
                        if inj and psrec_next[0] is not None:
                            psrec = psrec_next
                            psrec_next = [None, None]
                            inj = False  # pre-emitted 2 steps ago
                        sbq = s_local % PB
                        ab = [None, None]
                        tc_t = [None, None]
                        toks = [sbq, PB - 1 - sbq]
                        ts = [s, T - 1 - s]
                        p, pn = s % 2, (s + 1) % 2
                        # phase 1: both dirs' matmuls + gate tanh -> ctg[2:10]
                        for d in range(2):
                            if inj:
                                # gx injection emitted per-dir right before
                                # that dir's ladder, so dir0's ladder is not
                                # stuck behind dir1's 585ns inject on the PE
                                q = s_local // PB
                                psrec[d] = rps.tile([128, G, PB * BL], F32,
                                                    tag=f"ps{d}",
                                                    name=f"rps{d}")
                                tq = q if d == 0 else (RS // PB - 1) - q
                                nc.tensor.matmul(
                                    psrec[d][:], ident_sb[:],
                                    gx_cur[d][:, :, tq * 64:(tq + 1) * 64],
                                    start=True, stop=False,
                                    skip_group_check=True)
                            t, tok = ts[d], toks[d]
                            if s == 0:
                                hprev = hz
                            else:
                                tp = (t - 1) if d == 0 else (t + 1)
                                hprev = hb[layer][d][:, :, tp * BL:(tp + 1) * BL]
                            for k in range(KH):
                                for m in range(G):
                                    nc.tensor.matmul(
                                        psrec[d][:, m, tok * BL:(tok + 1) * BL],
                                        whh_sb[layer][d][:, k,
                                                         m * 128:(m + 1) * 128],
                                        hprev[:, k, :],
                                        start=False, stop=(k == KH - 1),
                                        skip_group_check=True)
                            nc.scalar.activation(
                                ctg[d][p][:, 2:10, :],
                                psrec[d][:, :, tok * BL:(tok + 1) * BL],
                                AF.Tanh)
                        # phase 2: fused cell updates on DVE.  gate chunk
                        # order is (g, f, i, o); ctg[0:2] = c~_prev, so
                        # ab = (ctg[4:8]+1)*ctg[0:4]
                        #    = [(f~+1)*c~_prev | (i~+1)*g~] = [a | b]
                        for d in range(2):
                            ab[d] = rec.tile([128, 4, BL], F32, tag=f"ab{d}",
                                             name=f"ab{d}")
                            nc.vector.scalar_tensor_tensor(
                                ab[d][:], ctg[d][p][:, 4:8, :], 1.0,
                                ctg[d][p][:, 0:4, :], ADD, MULT)
                            nc.vector.scalar_tensor_tensor(
                                ctg[d][pn][:, 0:2, :], ab[d][:, 0:2, :], 0.5,
                                ab[d][:, 2:4, :], MULT, ADD)
                        for d in range(2):
                            tc_t[d] = rec.tile([128, 2, BL], F32, tag=f"tc{d}",
                                               name=f"tc{d}")
                            nc.scalar.activation(tc_t[d][:],
                                                 ctg[d][pn][:, 0:2, :],
                                                 AF.Tanh, scale=0.5)
                        # h~ split by k-chunk: the mm ladder is k-outer, so
                        # the k=0 matmuls of step s+1 can start as soon as
                        # the first half of h~ lands.  (GpSimd/Pool rejects
                        # STT in this compiler build, so both stay on DVE.)
                        for d in range(2):
                            for k in range(KH):
                                nc.vector.scalar_tensor_tensor(
                                    hb[layer][d][:, k,
                                                 ts[d] * BL:(ts[d] + 1) * BL],
                                    ctg[d][p][:, 8 + k, :], 1.0,
                                    tc_t[d][:, k, :], ADD, MULT)
                        # gx/FC/DMA thunks drained at END of step so their
                        # ACT/DVE copies queue AFTER this step's chain ops.
                        # Rate-matched to the per-step PE idle gap (~1us):
                        # each mm thunk is a ~400ns matmul, so draining more
                        # than 2-3 per step overruns the gap and delays the
                        # next ladder past h-ready.
                        # gx injects for the NEXT PSUM block are emitted two
                        # steps early so the 585ns matmuls run in these
                        # steps' chain-wait gaps instead of delaying the
                        # block's first ladder (injects depend only on gx)
                        pre = (s_local % PB == PB - 2
                               and s_local // PB < RS // PB - 1)
                        if pre:
                            qq = s_local // PB + 1
                            for d in range(2):
                                psrec_next[d] = rps.tile(
                                    [128, G, PB * BL], F32,
                                    tag=f"ps{d}", name=f"rps{d}")
                                tq = qq if d == 0 else (RS // PB - 1) - qq
                                nc.tensor.matmul(
                                    psrec_next[d][:], ident_sb[:],
                                    gx_cur[d][:, :, tq * 64:(tq + 1) * 64],
                                    start=True, stop=False,
                                    skip_group_check=True)
                        for _ in range(3):
                            if thunks:
                                thunks.popleft()()
                        # cross-round: pre-emit the next round's block-0
                        # injects once this round's thunks have drained, so
                        # the round boundary doesn't serialize on them
                        if (s_local == RS - 2 and r + 1 < NR and not thunks
                                and gx_next[0] is not None):
                            for d in range(2):
                                psrec_next[d] = rps.tile(
                                    [128, G, PB * BL], F32,
                                    tag=f"ps{d}", name=f"rps{d}")
                                tq = 0 if d == 0 else RS // PB - 1
                                nc.tensor.matmul(
                                    psrec_next[d][:], ident_sb[:],
                                    gx_next[d][:, :, tq * 64:(tq + 1) * 64],
                                    start=True, stop=False,
                                    skip_group_check=True)
                    while thunks:
                        thunks.popleft()()
                    for d in range(2):
                        gx_cur[d] = gx_next[d]

        # FC epilogue: blocks finished in the last round
        for tb in (0, NR - 1):
            for f in fc_thunks(tb):
                f()

    return nc


# ---------------------------------------------------------------------------
# host-side preparation
# ---------------------------------------------------------------------------

def _rowscale():
    rs = np.full(4 * H, 0.5, np.float32)
    rs[2 * H:3 * H] = 1.0  # g rows keep scale 1 (direct tanh)
    return rs


# PyTorch gate order is (i, f, g, o) in 128-row chunks (0,1, 2,3, 4,5, 6,7);
# the kernel wants (g, f, i, o) so the fused DVE ab op sees contiguous
# [f, i] and [c, g] operand blocks.
_GPERM = np.array([4, 5, 2, 3, 0, 1, 6, 7])


def _permute_gates(wT):
    """Permute the last (1024-wide gate) axis of wT by _GPERM chunks."""
    shp = wT.shape
    v = wT.reshape(*shp[:-1], 8, 128)[..., _GPERM, :]
    return np.ascontiguousarray(v.reshape(*shp))


def _prep_w(wih, whh, b, din, in_half):
    """-> wihT [128, din/128, 1024] f16, whhT [128, 2, 1024], b [1, 1024] f16"""
    rs = _rowscale()
    kin = din // 128
    xs = 0.5 if in_half else 1.0
    wihT = (np.asarray(wih, np.float32) * rs[:, None] * xs).T
    wihT = _permute_gates(np.ascontiguousarray(wihT)).astype(np.float16)
    wihT = wihT.reshape(kin, 128, 1024).transpose(1, 0, 2)
    whhT = (np.asarray(whh, np.float32) * rs[:, None] * 0.5).T
    whhT = _permute_gates(np.ascontiguousarray(whhT))
    if WHH_F8:
        whhT = whhT.astype(ml_dtypes.float8_e4m3)
    else:
        whhT = whhT.astype(np.float16)
    whhT = whhT.reshape(2, 128, 1024).transpose(1, 0, 2)
    bp = np.ascontiguousarray((np.asarray(b, np.float32) * rs)
                              .reshape(8, 128)[_GPERM].T)  # [128, 8] chunks
    return (np.ascontiguousarray(wihT), np.ascontiguousarray(whhT), bp)


def prep_weight_map(inputs):
    m = {}
    w0 = [_prep_w(inputs["Wih_l0f"], inputs["Whh_l0f"], inputs["b_l0f"], D, False),
          _prep_w(inputs["Wih_l0b"], inputs["Whh_l0b"], inputs["b_l0b"], D, False)]
    w1 = [_prep_w(inputs["Wih_l1f"], inputs["Whh_l1f"], inputs["b_l1f"], 2 * H, True),
          _prep_w(inputs["Wih_l1b"], inputs["Whh_l1b"], inputs["b_l1b"], 2 * H, True)]
    m["wih0"] = np.stack([w0[0][0], w0[1][0]])
    m["whh0"] = np.stack([w0[0][1], w0[1][1]])
    m["b0"] = np.stack([w0[0][2], w0[1][2]])
    m["wih1"] = np.stack([w1[0][0], w1[1][0]])
    m["whh1"] = np.stack([w1[0][1], w1[1][1]])
    m["b1"] = np.stack([w1[0][2], w1[1][2]])
    fcT = (np.asarray(inputs["fc_W"], np.float32) * 0.5).T.astype(np.float16)
    m["fcw"] = np.ascontiguousarray(fcT.reshape(4, 128, 64).transpose(1, 0, 2))
    m["fcb"] = np.asarray(inputs["fc_b"], np.float32).reshape(64, 1)
    m["ident"] = np.eye(128, dtype=np.float16)
    return m


def prep_x_core(x, c, T):
    xs = np.asarray(x[c * BL:(c + 1) * BL, :T]).astype(np.float16)  # [8, T, 256]
    xt = xs.transpose(2, 1, 0).reshape(2, 128, T * BL).transpose(1, 0, 2)
    return np.ascontiguousarray(xt)


def run(inputs, T=1024, cores=None, trace=False):
    inputs = {k: np.asarray(v) for k, v in inputs.items()}
    if cores is None:
        cores = list(range(NCORES))
    nc = _patch_nc(build_nc(T))
    wm = prep_weight_map(inputs)
    in_maps = [dict(wm, xT=prep_x_core(inputs["x"], c, T)) for c in range(len(cores))]
    res = run_bass_kernel_spmd(nc, in_maps, core_ids=cores, trace=trace)
    outs = []
    for r in res.results:
        o = r["outT"].reshape(64, T, BL).transpose(2, 1, 0)  # [8, T, 64]
        outs.append(o)
    full = np.concatenate(outs, axis=0).astype(np.float32)
    return full, res


def kernel(**inputs):
    out, _ = run(inputs, T=1024, cores=list(range(NCORES)))
    return out

